# revision 14
# baseline (speedup 1.0000x reference)
"""Trainium2 Bass kernel for MixerNativeSparseAttention.

Sharding: one attention head per NeuronCore (8 heads / 8 cores). Each core
computes q/k/v/gate projections from the full sequence, compressed (pooled)
attention for its whole GQA group (needed for top-n block selection), then
top-8 block-sparse attention and sliding-window attention for its own head.
Host side only does layout prep (transpose/slice/replicate of inputs,
shape-derived tables/masks) and final concat of the 8 per-head outputs.
"""
import sys

for _p in ("/opt/trn_rl_repo",):
    if _p not in sys.path:
        sys.path.insert(0, _p)

import math
import numpy as np

import concourse.bass as bass
import concourse.tile as tile
from concourse import mybir
from concourse.bass_utils import run_bass_kernel_spmd

f32 = mybir.dt.float32
AF = mybir.ActivationFunctionType
OP = mybir.AluOpType
AX = mybir.AxisListType

B, T, DM = 1, 2048, 1024
H, KV, D = 8, 2, 128
G = H // KV
KS = 32
J = T // KS          # 64 blocks
TOPN = 8
WIN = 256
THETA = 10000.0
NEG = -1e30
BIG = 1e6
NT = T // 128        # 16 query tiles
NCH = DM // 128      # 8 dm chunks
SC = 512             # projection t-chunk width
NSC = T // SC        # 4


def _split_multi_waits(nc, max_waits=1):
    """walrus in this container rejects instructions carrying more than one
    semaphore wait; hoist extras onto injected same-engine NoOps just before
    the instruction (same sequencer stream => identical semantics)."""
    for fn in nc.m.functions:
        for bb in fn.blocks:
            out = []
            changed = False
            for inst in bb.instructions:
                si = inst.sync_info
                if si is not None and si.on_wait and len(si.on_wait) > max_waits:
                    waits = list(si.on_wait)
                    for k, w in enumerate(waits[max_waits:]):
                        nop = mybir.InstNoOp(
                            name=f"{inst.name}-sw{k}",
                            engine=inst.engine,
                            ins=[],
                            outs=[],
                            sync_info=mybir.SyncInfo(on_wait=[w], on_update=[]),
                        )
                        out.append(nop)
                    si.on_wait = waits[:max_waits]
                    changed = True
                out.append(inst)
            if changed:
                bb.instructions = out


def ts(i, n):
    return slice(i * n, (i + 1) * n)


def _build_program():
    # g_own == 0 on every core: the host permutes each core's wq so that the
    # core's own head occupies group slot 0 (the selection sum is order-
    # invariant across the GQA group).
    g_own = 0
    nc = bass.Bass("TRN2", target_bir_lowering=False, debug=False)

    din = {}
    for name, shape in [
        ("xT", [DM, T]),
        ("wq", [DM, G * D]),
        ("wk", [DM, D]),
        ("wv", [DM, D]),
        ("wg", [DM, 3]),
        ("qcos", [D, T]),
        ("qsin", [D, T]),
        ("perm", [D, D]),
        ("ident", [128, 128]),
        ("ccos", [J, D // 2]),
        ("csin", [J, D // 2]),
        ("wkbc", [128, KS]),
        ("wvbc", [128, KS]),
        ("wkpT", [KS, 1]),
        ("wvpT", [KS, 1]),
        ("pekv", [KS, D]),
        ("cmpm", [T, J]),
        ("amult", [T, J]),
        ("abias", [T, J]),
        ("pzero", [T, 1]),
        ("diagm", [128, 128]),
        ("swam", [128, 384]),
    ]:
        din[name] = nc.dram_tensor(name, shape, f32, kind="ExternalInput")
    o_d = nc.dram_tensor("o", [T, D], f32, kind="ExternalOutput")

    with tile.TileContext(nc) as tc:
        _emit(nc, tc, din, o_d, g_own)
    _split_multi_waits(nc)
    return nc


def _emit(nc, tc, din, o_d, g_own):
    from contextlib import ExitStack

    ctx = ExitStack()
    with ctx:
        # ---- persistent SBUF ----
        pp = ctx.enter_context(tc.tile_pool(name="pp", bufs=1))
        qTr = pp.tile([128, G, T], f32)      # roped q^T, 4 group heads
        kTr = pp.tile([128, T], f32)         # roped k^T (own kv head)
        kcT = pp.tile([128, J], f32)         # pooled (unroped) k^T
        vcT = pp.tile([128, J], f32)         # pooled v^T
        vnat = pp.tile([128, NT, 128], f32)  # v natural, tile b at [:, b, :]
        gnat = pp.tile([128, NT, 3], f32)    # sigmoid gates per query
        pown = pp.tile([128, NT, J], f32)    # own-head compressed probs
        selad = pp.tile([128, NT, J], f32)   # additive top-8 select mask
        kcrT = pp.tile([128, J], f32)        # roped compressed k^T
        vc = pp.tile([J, D], f32)            # compressed v (natural)

        cst = ctx.enter_context(tc.tile_pool(name="cst", bufs=1))
        ident = cst.tile([128, 128], f32)
        nc.sync.dma_start(ident[:], din["ident"].ap())
        perm = cst.tile([D, D], f32)
        nc.sync.dma_start(perm[:], din["perm"].ap())
        ccos = cst.tile([J, D // 2], f32)
        nc.sync.dma_start(ccos[:], din["ccos"].ap())
        csin = cst.tile([J, D // 2], f32)
        nc.sync.dma_start(csin[:], din["csin"].ap())
        wkbc = cst.tile([128, KS], f32)
        nc.sync.dma_start(wkbc[:], din["wkbc"].ap())
        wvbc = cst.tile([128, KS], f32)
        nc.sync.dma_start(wvbc[:], din["wvbc"].ap())
        wkpT = cst.tile([KS, 1], f32)
        nc.sync.dma_start(wkpT[:], din["wkpT"].ap())
        wvpT = cst.tile([KS, 1], f32)
        nc.sync.dma_start(wvpT[:], din["wvpT"].ap())
        pekv = cst.tile([KS, D], f32)
        nc.sync.dma_start(pekv[:], din["pekv"].ap())
        cmpm = cst.tile([128, NT, J], f32)
        nc.sync.dma_start(cmpm[:], din["cmpm"].ap().rearrange("(n p) j -> p n j", p=128))
        amult = cst.tile([128, NT, J], f32)
        nc.sync.dma_start(amult[:], din["amult"].ap().rearrange("(n p) j -> p n j", p=128))
        abias = cst.tile([128, NT, J], f32)
        nc.sync.dma_start(abias[:], din["abias"].ap().rearrange("(n p) j -> p n j", p=128))
        pzero = cst.tile([128, NT], f32)
        nc.sync.dma_start(pzero[:], din["pzero"].ap().rearrange("(n p) o -> p (n o)", p=128))
        diagm = cst.tile([128, 128], f32)
        nc.sync.dma_start(diagm[:], din["diagm"].ap())
        swam = cst.tile([128, 384], f32)
        nc.sync.dma_start(swam[:], din["swam"].ap())

        # ================= stage A: projections + rope =================
        with (
            tc.tile_pool(name="pa", bufs=1) as pa,
            tc.tile_pool(name="psA", bufs=1, space="PSUM") as psA,
            tc.tile_pool(name="psB", bufs=1, space="PSUM") as psB,
            tc.tile_pool(name="wsA", bufs=3) as wsA,
            tc.tile_pool(name="px", bufs=4) as px,
        ):
            wq = pa.tile([128, NCH, G * D], f32)
            nc.sync.dma_start(wq[:], din["wq"].ap().rearrange("(c p) n -> p c n", p=128))
            wk = pa.tile([128, NCH, D], f32)
            nc.sync.dma_start(wk[:], din["wk"].ap().rearrange("(c p) n -> p c n", p=128))
            wv = pa.tile([128, NCH, D], f32)
            nc.sync.dma_start(wv[:], din["wv"].ap().rearrange("(c p) n -> p c n", p=128))
            wg = pa.tile([128, NCH, 3], f32)
            nc.sync.dma_start(wg[:], din["wg"].ap().rearrange("(c p) n -> p c n", p=128))
            qcos = pa.tile([128, T], f32)
            nc.sync.dma_start(qcos[:], din["qcos"].ap())
            qsin = pa.tile([128, T], f32)
            nc.sync.dma_start(qsin[:], din["qsin"].ap())

            for tch in range(NSC):
                col = ts(tch, SC)
                qt_ps = [
                    psA.tile([128, SC], f32, tag=f"qt{gh}", name=f"qt_ps{gh}")
                    for gh in range(G)
                ]
                kt_ps = psA.tile([128, SC], f32, tag="kt")
                vt_ps = psA.tile([128, SC], f32, tag="vt")
                gt_ps = psA.tile([3, SC], f32, tag="gt")
                for dm in range(NCH):
                    xsl = px.tile([128, SC], f32, tag="xsl")
                    nc.sync.dma_start(xsl[:], din["xT"].ap()[ts(dm, 128), col])
                    rhs = xsl[:]
                    st, sp = dm == 0, dm == NCH - 1
                    for gh in range(G):
                        nc.tensor.matmul(qt_ps[gh][:], wq[:, dm, ts(gh, D)], rhs, start=st, stop=sp)
                    nc.tensor.matmul(kt_ps[:], wk[:, dm, :], rhs, start=st, stop=sp)
                    nc.tensor.matmul(vt_ps[:], wv[:, dm, :], rhs, start=st, stop=sp)
                    nc.tensor.matmul(gt_ps[:], wg[:, dm, :], rhs, start=st, stop=sp)

                # k/v evacuation (per-chunk transients)
                ktc = wsA.tile([128, SC], f32, tag="ktc")
                nc.scalar.copy(ktc[:], kt_ps[:])
                vtc = wsA.tile([128, SC], f32, tag="vtc")
                nc.vector.tensor_copy(vtc[:], vt_ps[:])

                # pooled k^T / v^T for this chunk's 16 blocks
                for src, dst, wbc in ((ktc, kcT, wkbc), (vtc, vcT, wvbc)):
                    ptmp = wsA.tile([128, SC], f32, tag="pooltmp")
                    nc.vector.tensor_tensor(
                        ptmp[:].rearrange("p (a c) -> p a c", c=KS),
                        src[:].rearrange("p (a c) -> p a c", c=KS),
                        wbc[:].unsqueeze(1).broadcast_to([128, SC // KS, KS]),
                        OP.mult,
                    )
                    nc.vector.reduce_sum(
                        dst[:, ts(tch, SC // KS)],
                        ptmp[:].rearrange("p (a c) -> p a c", c=KS),
                        axis=AX.X,
                    )

                # rope(k): kTr = kT*cos + (perm @ kT)*sin
                pk_ps = psB.tile([128, SC], f32, tag="rot")
                nc.tensor.matmul(pk_ps[:], perm[:], ktc[:], start=True, stop=True)
                tmps = wsA.tile([128, SC], f32, tag="ropetmp")
                nc.vector.tensor_tensor(tmps[:], pk_ps[:], qsin[:, col], OP.mult)
                nc.vector.tensor_tensor(kTr[:, col], ktc[:], qcos[:, col], OP.mult)
                nc.vector.tensor_tensor(kTr[:, col], kTr[:, col], tmps[:], OP.add)

                # rope(q) for the 4 group heads
                for gh in range(G):
                    qtmp = wsA.tile([128, SC], f32, tag="qtmp")
                    nc.scalar.copy(qtmp[:], qt_ps[gh][:])
                    pq_ps = psB.tile([128, SC], f32, tag="rot")
                    nc.tensor.matmul(pq_ps[:], perm[:], qtmp[:], start=True, stop=True)
                    tmpq = wsA.tile([128, SC], f32, tag="ropetmp")
                    nc.vector.tensor_tensor(tmpq[:], pq_ps[:], qsin[:, col], OP.mult)
                    nc.vector.tensor_tensor(qTr[:, gh, col], qtmp[:], qcos[:, col], OP.mult)
                    nc.vector.tensor_tensor(qTr[:, gh, col], qTr[:, gh, col], tmpq[:], OP.add)

                # v natural via PE transpose
                for s in range(SC // 128):
                    b = tch * (SC // 128) + s
                    tr_ps = psB.tile([128, SC], f32, tag="rot")
                    nc.tensor.transpose(tr_ps[:, 0:128], vtc[:, ts(s, 128)], ident[:])
                    nc.vector.tensor_copy(vnat[:, b, :], tr_ps[:, 0:128])

                # gates: transpose + sigmoid
                gts = wsA.tile([3, SC], f32, tag="gts")
                nc.scalar.copy(gts[:], gt_ps[:])
                for s in range(SC // 128):
                    b = tch * (SC // 128) + s
                    gn_ps = psB.tile([128, SC], f32, tag="rot")
                    nc.tensor.transpose(gn_ps[:, 0:3], gts[0:3, ts(s, 128)], ident[0:3, 0:3])
                    nc.scalar.activation(gnat[:, b, :], gn_ps[:, 0:3], AF.Sigmoid)

        # ================= stage B: compressed attn + selection =================
        with (
            tc.tile_pool(name="psC", bufs=2, space="PSUM") as psC,
            tc.tile_pool(name="wsB", bufs=2) as wsB,
            tc.tile_pool(name="wsBs", bufs=4) as wsBs,
        ):
            # pe pooled: pe_k[d] = sum_s wk_pool[s]*pe[s,d], as [128,1] per-partition
            pek_ps = psC.tile([128, 512], f32, tag="sc")
            nc.tensor.matmul(pek_ps[0:1, 0:D], wkpT[:], pekv[:], start=True, stop=True)
            pek_row = wsB.tile([1, D], f32, tag="perow")
            nc.vector.tensor_copy(pek_row[:], pek_ps[0:1, 0:D])
            pekT_ps = psC.tile([128, 512], f32, tag="sc")
            nc.tensor.transpose(pekT_ps[:, 0:1], pek_row[0:1, :], ident[0:1, 0:1])
            pekT = wsB.tile([128, 1], f32, tag="pekT")
            nc.vector.tensor_copy(pekT[:], pekT_ps[:, 0:1])

            pev_ps = psC.tile([128, 512], f32, tag="sc")
            nc.tensor.matmul(pev_ps[0:1, 0:D], wvpT[:], pekv[:], start=True, stop=True)
            pev_row = wsB.tile([1, D], f32, tag="perow")
            nc.vector.tensor_copy(pev_row[:], pev_ps[0:1, 0:D])
            pevT_ps = psC.tile([128, 512], f32, tag="sc")
            nc.tensor.transpose(pevT_ps[:, 0:1], pev_row[0:1, :], ident[0:1, 0:1])
            pevT = wsB.tile([128, 1], f32, tag="pekT")
            nc.vector.tensor_copy(pevT[:], pevT_ps[:, 0:1])

            # pe offsets onto the pooled k^T / v^T
            nc.vector.tensor_scalar(kcT[:], kcT[:], pekT[:, 0:1], None, OP.add)
            nc.vector.tensor_scalar(vcT[:], vcT[:], pevT[:, 0:1], None, OP.add)

            # rope(k_cmp) in natural layout, then back to [128, J]
            kcn_ps = psC.tile([128, 512], f32, tag="sc")
            nc.tensor.transpose(kcn_ps[:J, 0:D], kcT[:], ident[:])
            kcn = wsB.tile([J, D], f32, tag="kcn")
            nc.vector.tensor_copy(kcn[:], kcn_ps[:J, 0:D])
            kcr = wsB.tile([J, D], f32, tag="kcr")
            half = D // 2
            tmpc = wsB.tile([J, half], f32, tag="tmpc")
            nc.vector.tensor_tensor(kcr[:, 0:half], kcn[:, 0:half], ccos[:], OP.mult)
            nc.vector.tensor_tensor(tmpc[:], kcn[:, half:D], csin[:], OP.mult)
            nc.vector.tensor_tensor(kcr[:, 0:half], kcr[:, 0:half], tmpc[:], OP.subtract)
            nc.vector.tensor_tensor(kcr[:, half:D], kcn[:, half:D], ccos[:], OP.mult)
            nc.vector.tensor_tensor(tmpc[:], kcn[:, 0:half], csin[:], OP.mult)
            nc.vector.tensor_tensor(kcr[:, half:D], kcr[:, half:D], tmpc[:], OP.add)
            kcrT_ps = psC.tile([128, 512], f32, tag="sc")
            nc.tensor.transpose(kcrT_ps[:, 0:J], kcr[:], ident[0:J, 0:J])
            nc.vector.tensor_copy(kcrT[:], kcrT_ps[:, 0:J])

            # v_cmp natural
            vcn_ps = psC.tile([128, 512], f32, tag="sc")
            nc.tensor.transpose(vcn_ps[:J, 0:D], vcT[:], ident[:])
            nc.vector.tensor_copy(vc[:], vcn_ps[:J, 0:D])

            # compressed attention for all 4 group heads + top-8 selection
            for i in range(NT):
                pw = wsBs.tile([128, J], f32, tag="pw")
                for gh in range(G):
                    sc_ps = psC.tile([128, 512], f32, tag="sc")
                    nc.tensor.matmul(sc_ps[:, 0:J], qTr[:, gh, ts(i, 128)], kcrT[:], start=True, stop=True)
                    nc.vector.tensor_tensor(sc_ps[:, 0:J], sc_ps[:, 0:J], cmpm[:, i, :], OP.add)
                    nm = wsBs.tile([128, 1], f32, tag="nm")
                    nc.vector.reduce_max(nm[:], sc_ps[:, 0:J], axis=AX.X, negate=True)
                    pt = wsBs.tile([128, J], f32, tag="pt")
                    den = wsBs.tile([128, 1], f32, tag="den")
                    nc.scalar.activation(pt[:], sc_ps[:, 0:J], AF.Exp, bias=nm[:], accum_out=den[:])
                    rc = wsBs.tile([128, 1], f32, tag="rc")
                    nc.vector.reciprocal(rc[:], den[:])
                    if gh == 0:
                        nc.vector.tensor_scalar(pw[:], pt[:], rc[:, 0:1], None, OP.mult)
                    else:
                        nc.vector.scalar_tensor_tensor(pw[:], pt[:], rc[:, 0:1], pw[:], OP.mult, OP.add)
                    if gh == g_own:
                        nc.vector.tensor_scalar(
                            pown[:, i, :], pt[:], rc[:, 0:1], pzero[:, i : i + 1], OP.mult, OP.mult
                        )
                score = wsBs.tile([128, J], f32, tag="score")
                nc.vector.tensor_tensor(score[:], pw[:], amult[:, i, :], OP.mult)
                nc.vector.tensor_tensor(score[:], score[:], abias[:, i, :], OP.add)
                m8 = wsBs.tile([128, 8], f32, tag="m8")
                nc.vector.max(m8[:], score[:])
                selm = wsBs.tile([128, J], f32, tag="selm")
                nc.vector.tensor_scalar(selm[:], score[:], m8[:, 7:8], None, OP.is_ge)
                nc.vector.tensor_tensor(selm[:], selm[:], amult[:, i, :], OP.mult)
                nc.vector.tensor_scalar(selad[:, i, :], selm[:], 1e30, -1e30, OP.mult, OP.add)

        # ================= stage C: slc + swa attention =================
        with (
            tc.tile_pool(name="psS", bufs=2, space="PSUM") as psS,
            tc.tile_pool(name="psO", bufs=1, space="PSUM") as psO,
            tc.tile_pool(name="psT", bufs=2, space="PSUM") as psT,
            tc.tile_pool(name="wsC", bufs=2) as wsC,
            tc.tile_pool(name="wsCs", bufs=4) as wsCs,
        ):
            for i in range(NT):
                nb = i + 1
                ncol = nb * 128
                ngr = (ncol + 511) // 512
                qsl = qTr[:, g_own, ts(i, 128)]

                pm = wsC.tile([128, T], f32, tag="pm")
                s_ps = []
                for gi in range(ngr):
                    c0 = gi * 512
                    cw = min(512, ncol - c0)
                    sp = psS.tile([128, 512], f32, tag="sg")
                    nc.tensor.matmul(sp[:, 0:cw], qsl, kTr[:, c0 : c0 + cw], start=True, stop=True)
                    s_ps.append((sp, c0, cw))
                    # slc additive mask: broadcast selad (j-blocks) + add
                    nj = cw // KS
                    j0 = c0 // KS
                    nc.vector.scalar_tensor_tensor(
                        pm[:, c0 : c0 + cw].rearrange("p (a c) -> p a c", c=KS),
                        sp[:, 0:cw].rearrange("p (a c) -> p a c", c=KS),
                        1.0,
                        selad[:, i, j0 : j0 + nj].unsqueeze(2).broadcast_to([128, nj, KS]),
                        OP.bypass,
                        OP.add,
                    )
                # causal triangle on the diagonal tile
                nc.vector.tensor_tensor(
                    pm[:, ncol - 128 : ncol], pm[:, ncol - 128 : ncol], diagm[:], OP.add
                )

                # ---- swa band mask (reads raw scores from psum groups) ----
                b0 = max(0, i - 2)
                w = (nb - b0) * 128
                pmw = wsCs.tile([128, 384], f32, tag="pmw")
                for sp, c0, cw in s_ps:
                    lo = max(b0 * 128, c0)
                    hi = min(ncol, c0 + cw)
                    if lo >= hi:
                        continue
                    rel = lo - b0 * 128
                    nc.vector.scalar_tensor_tensor(
                        pmw[:, rel : rel + hi - lo],
                        sp[:, lo - c0 : hi - c0],
                        1.0,
                        swam[:, 384 - w + rel : 384 - w + rel + hi - lo],
                        OP.bypass,
                        OP.add,
                    )

                # ---- slc softmax + PV ----
                nm = wsCs.tile([128, 1], f32, tag="nm")
                nc.vector.reduce_max(nm[:], pm[:, 0:ncol], axis=AX.X, negate=True)
                pe_t = wsC.tile([128, T], f32, tag="pe")
                den = wsCs.tile([128, 1], f32, tag="den")
                nc.scalar.activation(pe_t[:, 0:ncol], pm[:, 0:ncol], AF.Exp, bias=nm[:], accum_out=den[:])
                rcs = wsCs.tile([128, 1], f32, tag="rcs")
                nc.vector.reciprocal(rcs[:], den[:])
                o_slc = psO.tile([128, 128], f32, tag="oslc")
                for b in range(nb):
                    tp = psT.tile([128, 128], f32, tag="tp")
                    nc.tensor.transpose(tp[:], pe_t[:, ts(b, 128)], ident[:])
                    ptT = wsCs.tile([128, 128], f32, tag="ptT")
                    (nc.scalar.copy if b % 2 else nc.vector.tensor_copy)(ptT[:], tp[:])
                    nc.tensor.matmul(o_slc[:], ptT[:], vnat[:, b, :], start=(b == 0), stop=(b == nb - 1))

                # ---- swa softmax + PV ----
                nmw = wsCs.tile([128, 1], f32, tag="nmw")
                nc.vector.reduce_max(nmw[:], pmw[:, 0:w], axis=AX.X, negate=True)
                pew = wsCs.tile([128, 384], f32, tag="pew")
                denw = wsCs.tile([128, 1], f32, tag="denw")
                nc.scalar.activation(pew[:, 0:w], pmw[:, 0:w], AF.Exp, bias=nmw[:], accum_out=denw[:])
                rcw = wsCs.tile([128, 1], f32, tag="rcw")
                nc.vector.reciprocal(rcw[:], denw[:])
                o_swa = psO.tile([128, 128], f32, tag="oswa")
                for b in range(b0, nb):
                    tp = psT.tile([128, 128], f32, tag="tp")
                    nc.tensor.transpose(tp[:], pew[:, ts(b - b0, 128)], ident[:])
                    ptT = wsCs.tile([128, 128], f32, tag="ptT")
                    (nc.scalar.copy if b % 2 else nc.vector.tensor_copy)(ptT[:], tp[:])
                    nc.tensor.matmul(o_swa[:], ptT[:], vnat[:, b, :], start=(b == b0), stop=(b == nb - 1))

                # ---- compressed PV (already normalized) ----
                tpc = psT.tile([128, 128], f32, tag="tp")
                nc.tensor.transpose(tpc[:J, :], pown[:, i, :], ident[:])
                pcT = wsCs.tile([J, 128], f32, tag="pcT")
                nc.vector.tensor_copy(pcT[:], tpc[:J, :])
                o_cmp = psO.tile([128, 128], f32, tag="ocmp")
                nc.tensor.matmul(o_cmp[:], pcT[:], vc[:], start=True, stop=True)

                # ---- gated combine + store ----
                gsl = wsCs.tile([128, 1], f32, tag="gsl")
                nc.vector.tensor_tensor(gsl[:], gnat[:, i, 1:2], rcs[:], OP.mult)
                gsw = wsCs.tile([128, 1], f32, tag="gsw")
                nc.vector.tensor_tensor(gsw[:], gnat[:, i, 2:3], rcw[:], OP.mult)
                outt = wsCs.tile([128, 128], f32, tag="outt")
                nc.vector.tensor_scalar(outt[:], o_cmp[:], gnat[:, i, 0:1], None, OP.mult)
                nc.vector.scalar_tensor_tensor(outt[:], o_slc[:], gsl[:, 0:1], outt[:], OP.mult, OP.add)
                nc.vector.scalar_tensor_tensor(outt[:], o_swa[:], gsw[:, 0:1], outt[:], OP.mult, OP.add)
                nc.sync.dma_start(o_d.ap()[ts(i, 128), :], outt[:])


# ---------------- host side ----------------

def _host_tables():
    half = D // 2
    inv = 1.0 / (THETA ** (np.arange(half, dtype=np.float64) / half))
    tpos = np.arange(T, dtype=np.float64)
    ang = tpos[None, :] * inv[:, None]            # [half, T]
    qcos = np.concatenate([np.cos(ang), np.cos(ang)], 0).astype(np.float32)
    qsin = np.concatenate([np.sin(ang), np.sin(ang)], 0).astype(np.float32)
    perm = np.zeros((D, D), np.float32)           # lhsT: out[m] = sum_k perm[k,m] in[k]
    for m in range(half):
        perm[m + half, m] = -1.0
    for m in range(half, D):
        perm[m - half, m] = 1.0
    bang = (np.arange(J, dtype=np.float64) * KS)[:, None] * inv[None, :]  # [J, half]
    ccos = np.cos(bang).astype(np.float32)
    csin = np.sin(bang).astype(np.float32)

    pos = np.arange(T)
    blk = np.arange(J)
    vis = pos[:, None] >= (blk[None, :] * KS + KS - 1)
    cmpm = np.where(vis, 0.0, NEG).astype(np.float32)
    pzero = vis.any(-1).astype(np.float32)[:, None]
    cur = pos // KS
    forced = (blk[None, :] == 0) | (blk[None, :] == cur[:, None]) | (
        blk[None, :] == np.maximum(cur - 1, 0)[:, None]
    )
    allowed = (blk[None, :] * KS) <= pos[:, None]
    amult = allowed.astype(np.float32)
    abias = np.where(allowed, BIG * forced, -1.0).astype(np.float32)

    r = np.arange(128)
    c = np.arange(128)
    diagm = np.where(c[None, :] <= r[:, None], 0.0, NEG).astype(np.float32)
    cb = np.arange(384)
    swam = np.where((cb[None, :] >= r[:, None]) & (cb[None, :] <= r[:, None] + WIN), 0.0, NEG).astype(np.float32)
    return dict(
        qcos=qcos, qsin=qsin, perm=perm, ccos=ccos, csin=csin, cmpm=cmpm,
        pzero=pzero, amult=amult, abias=abias, diagm=diagm, swam=swam,
        ident=np.eye(128, dtype=np.float32),
    )


def _host_in_maps(inputs):
    x = np.asarray(inputs["x"], np.float32)
    Wq = np.asarray(inputs["Wq"], np.float32)
    Wk = np.asarray(inputs["Wk"], np.float32)
    Wv = np.asarray(inputs["Wv"], np.float32)
    Wg = np.asarray(inputs["Wg"], np.float32)
    wk_pool = np.asarray(inputs["wk_pool"], np.float32)
    wv_pool = np.asarray(inputs["wv_pool"], np.float32)
    pe = np.asarray(inputs["pe"], np.float32)

    tables = _host_tables()
    xT = np.ascontiguousarray(x[0].T)
    scale = 1.0 / math.sqrt(D)
    maps = []
    for h in range(H):
        kv, g = h // G, h % G
        m = dict(tables)
        m["xT"] = xT
        wq_grp = Wq[:, kv * G * D : (kv + 1) * G * D].reshape(DM, G, D)
        order = [g] + [x for x in range(G) if x != g]
        m["wq"] = np.ascontiguousarray(wq_grp[:, order, :].reshape(DM, G * D)) * scale
        m["wk"] = np.ascontiguousarray(Wk[:, kv * D : (kv + 1) * D])
        m["wv"] = np.ascontiguousarray(Wv[:, kv * D : (kv + 1) * D])
        m["wg"] = np.ascontiguousarray(Wg[:, kv * G * 3 + g * 3 : kv * G * 3 + g * 3 + 3])
        m["wkbc"] = np.tile(wk_pool[kv][None, :], (128, 1))
        m["wvbc"] = np.tile(wv_pool[kv][None, :], (128, 1))
        m["wkpT"] = np.ascontiguousarray(wk_pool[kv][:, None])
        m["wvpT"] = np.ascontiguousarray(wv_pool[kv][:, None])
        m["pekv"] = np.ascontiguousarray(pe[kv])
        maps.append(m)
    return maps


_CACHE = {}


def _get_program():
    if "prog" not in _CACHE:
        _CACHE["prog"] = _build_program()
    return _CACHE["prog"]


def kernel(**inputs) -> np.ndarray:
    maps = _host_in_maps(inputs)
    nc = _get_program()
    res = run_bass_kernel_spmd(nc, maps, list(range(H)))
    o = np.concatenate([res.results[h]["o"] for h in range(H)], axis=-1)
    return o.reshape(B, T, H * D).astype(np.float32)
